# revision 2
# baseline (speedup 1.0000x reference)
"""4-bit column-block-quantized linear on 8 TRN2 cores — fp8 DoubleRow, v2.

Math:  out[b,o] = scales[o] * (sum_i inp[b,i]*wq[o,i] - zeros[o]*rowsum[b])
where wq nibbles come from packed bytes q[o,j] (j = i//2): even i -> low
nibble, odd i -> high nibble.

Device scheme (all O(O*I) work on-device):
  * Packed bytes stream through the PE as float8e4: nibble bit patterns
    0x0..0xF ARE e4m3 values nibble*2^-9, so unpacking is 2 DVE
    tensor_scalar ops per chunk (uint32 views, 2x_2p mode):
        l = q & 0x0F0F0F0F ; h = (q >> 4) & 0x0F0F0F0F
    The 2^9 folds into the final host-side scales multiply.
  * fp8 DoubleRow matmuls: stationary = activations split hi/lo fp8
    (psum rows 0:16 hi, 16:32 lo), moving = the nibble streams.
  * -zeros*rowsum lands via a K=4 bf16 rank-1 matmul issued first, plus
    warm-up matmuls into a scratch psum bank so the PE p-state ramps
    while the weight DMAs stream in.
  * Drain: DVE copies psum[0:32] to SBUF, one DMA out; host adds the
    hi/lo planes and applies 512*scales.

Layout/overlap:
  * q repacked host-side to partition-contiguous [128, 22016B] so the
    weight stream needs only 5 big DMAs (1+1+2+2+2 dkt chunks), issued
    back-to-back on Sync while Scalar issues the const DMAs in parallel.
  * No buffer reuse anywhere (single-assignment tiles) to minimize
    semaphores and anti-dependency stalls.

Sharding: column-parallel over out_features (1376 rows/core), inputs
replicated; per-core output [2*16,1376] gathered+reduced on host.
"""

import numpy as np
import ml_dtypes

B = 16
I = 4096
O = 11008
NCORES = 8
OS = O // NCORES          # 1376 out-features per core
HALF = I // 2             # 2048 packed columns (j)
NDKT = 8                  # double-k-tiles of 256 j-rows each
BLKS = [(0, 512), (512, 512), (1024, 352)]  # psum-bank o-blocks
CHUNKS = [(0, 1), (1, 1), (2, 2), (4, 2), (6, 2)]  # q DMA chunks (d0, ndkt)
NWARM = 2                 # PE warm-up matmuls into scratch psum

BF16 = ml_dtypes.bfloat16
FP8 = ml_dtypes.float8_e4m3fn

_CACHE = {}


def _split_bf16(x64):
    hi = x64.astype(BF16)
    lo = (x64 - hi.astype(np.float64)).astype(BF16)
    return hi, lo


def _split_fp8(x64):
    hi = x64.astype(FP8)
    lo = (x64 - hi.astype(np.float64)).astype(FP8)
    return hi, lo


def _build_program():
    import concourse.bacc as bacc
    import concourse.mybir as mybir
    import concourse.tile as tile

    dt = mybir.dt
    op = mybir.AluOpType
    pm = mybir.MatmulPerfMode
    nc = bacc.Bacc("TRN2", target_bir_lowering=False)

    qa = nc.dram_tensor("qa", [128, NDKT * 688], dt.uint32, kind="ExternalInput")
    stat = nc.dram_tensor(
        "stat", [128, NDKT * 2 * 64], dt.float8e4, kind="ExternalInput"
    )
    corr = nc.dram_tensor("corr", [4, 32 + OS], dt.bfloat16, kind="ExternalInput")
    out_d = nc.dram_tensor("out", [32, OS], dt.float32, kind="ExternalOutput")

    with tile.TileContext(nc) as tc:
        with (
            tc.tile_pool(name="p", bufs=1) as pool,
            tc.tile_pool(name="ps", bufs=1, space="PSUM") as pspool,
        ):
            stat_sb = pool.tile([128, NDKT * 128], dt.float8e4, name="stat_sb")
            corr_sb = pool.tile([4, 32 + OS], dt.bfloat16, name="corr_sb")
            corrL = corr_sb[:, 0:32]
            corrR = corr_sb[:, 32 : 32 + OS]
            qts = [
                pool.tile([128, 688 * n], dt.uint32, name=f"qt{c}")
                for c, (d0, n) in enumerate(CHUNKS)
            ]
            lbs = [
                pool.tile([128, 688 * n], dt.uint32, name=f"lb{c}")
                for c, (d0, n) in enumerate(CHUNKS)
            ]
            hbs = [
                pool.tile([128, 688 * n], dt.uint32, name=f"hb{c}")
                for c, (d0, n) in enumerate(CHUNKS)
            ]
            out_sb = pool.tile([32, OS], dt.float32, name="out_sb")

            psums = [
                pspool.tile([32, n], dt.float32, name=f"ps{i}")
                for i, (s, n) in enumerate(BLKS)
            ]
            ps_w = pspool.tile([32, 512], dt.float32, name="psw")

            # consts on Scalar's HWDGE, weight chunks on Sync's — parallel issue
            nc.scalar.dma_start(corr_sb, corr[:, :])
            nc.scalar.dma_start(stat_sb, stat[:, :])
            for c, (d0, n) in enumerate(CHUNKS):
                nc.sync.dma_start(qts[c], qa[:, d0 * 688 : (d0 + n) * 688])

            # rank-1 correction first (PE busy during DMA fill), then
            # warm-ups to keep the p-state ramp alive until dkt0 is ready
            for i, (s, n) in enumerate(BLKS):
                nc.tensor.matmul(
                    psums[i], corrL, corrR[:, s : s + n], start=True, stop=False
                )
            for _ in range(NWARM):
                nc.tensor.matmul(ps_w, corrL, corrR[:, 0:512], start=True, stop=True)

            def stat_ap(d, s):
                a = stat_sb[:, d * 128 + s * 64 : d * 128 + (s + 1) * 64]
                return a.rearrange("p (g m) -> p g m", g=2)

            for c, (d0, ndk) in enumerate(CHUNKS):
                lb, hb, qt = lbs[c], hbs[c], qts[c]
                nc.vector.tensor_scalar(lb, qt, 0x0F0F0F0F, None, op.bitwise_and)
                nc.vector.tensor_scalar(
                    hb, qt, 4, 0x0F0F0F0F, op.logical_shift_right, op.bitwise_and
                )
                for t in range(ndk):
                    d = d0 + t
                    last = d == NDKT - 1
                    for s, buf in ((0, lb), (1, hb)):
                        mv = (
                            buf.bitcast(dt.float8e4)[:, t * 2752 : (t + 1) * 2752]
                            .rearrange("p (o g) -> p g o", g=2)
                        )
                        sa = stat_ap(d, s)
                        for i, (s0, n) in enumerate(BLKS):
                            stop = last and s == 1
                            nc.tensor.matmul(
                                psums[i], sa, mv[:, :, s0 : s0 + n],
                                start=False, stop=stop, perf_mode=pm.DoubleRow,
                            )
                            if stop:
                                # psum hi+lo rows -> SBUF; host adds the planes
                                nc.vector.tensor_scalar(
                                    out_sb[:, s0 : s0 + n], psums[i], 0.0, None, op.add
                                )
            nc.sync.dma_start(out_d[:, :], out_sb)

    nc.finalize()
    return nc


def _get_program():
    if "nc" not in _CACHE:
        _CACHE["nc"] = _build_program()
    return _CACHE["nc"]


def _host_prep(inp, quant_weight, scales, zeros):
    """Build per-core input maps (layout/precision prep, no dequant math)."""
    inp64 = np.asarray(inp, dtype=np.float64)
    a = np.ascontiguousarray(inp64[:, 0::2].T)  # [HALF, B] even-i (pairs l)
    b = np.ascontiguousarray(inp64[:, 1::2].T)  # [HALF, B] odd-i  (pairs h)
    a_hi, a_lo = _split_fp8(a)
    b_hi, b_lo = _split_fp8(b)

    def stream_stat(hi, lo):
        # [HALF,B] -> [NDKT,2,128,2B]: per dkt d, group g, j=d*256+g*128+p,
        # cols [hi(16) lo(16)]
        h = hi.reshape(NDKT, 2, 128, B)
        l = lo.reshape(NDKT, 2, 128, B)
        return np.concatenate([h, l], axis=-1)  # [d, g, p, 32]

    sa = stream_stat(a_hi, a_lo)  # stream 0: even i
    sb = stream_stat(b_hi, b_lo)  # stream 1: odd i
    st = np.stack([sa, sb], axis=1)  # [d, s, g, p, 32]
    stat_m = np.ascontiguousarray(
        st.transpose(3, 0, 1, 2, 4).reshape(128, NDKT * 2 * 2 * 32)
    )

    rowsum = inp64.sum(axis=1)  # [B]
    rs_hi, rs_lo = _split_bf16(rowsum)
    s9 = np.float64(2.0**-9)
    corrL = np.zeros((4, 32), dtype=BF16)
    corrL[0, :B] = (rs_hi.astype(np.float64) * s9).astype(BF16)
    corrL[1, :B] = corrL[0, :B]
    corrL[2, :B] = (rs_lo.astype(np.float64) * s9).astype(BF16)
    corrL[3, :B] = corrL[2, :B]

    qw = np.asarray(quant_weight)
    zeros = np.asarray(zeros, dtype=np.float64).reshape(-1)

    in_maps = []
    for cidx in range(NCORES):
        rows = slice(cidx * OS, (cidx + 1) * OS)
        qc = np.ascontiguousarray(qw[rows].astype(np.uint8).T)  # [HALF, OS]
        # byte (2o+g) of (dkt d, partition p) = qc[d*256+g*128+p, o];
        # partition-contiguous: partition p holds dkt d at byte offset d*2752
        q_arr = np.ascontiguousarray(
            qc.reshape(NDKT, 2, 128, OS)
            .transpose(2, 0, 3, 1)  # [p, d, o, g]
            .reshape(128, NDKT * 2 * OS)
        ).view(np.uint32)
        z_hi, z_lo = _split_bf16(zeros[rows])
        corr_m = np.zeros((4, 32 + OS), dtype=BF16)
        corr_m[:, :32] = corrL
        corr_m[0, 32:] = -z_hi
        corr_m[1, 32:] = -z_lo
        corr_m[2, 32:] = -z_hi
        corr_m[3, 32:] = -z_lo
        in_maps.append({"qa": q_arr, "stat": stat_m, "corr": corr_m})
    return in_maps


def kernel(inp, quant_weight, scales, zeros):
    from concourse.bass_utils import run_bass_kernel_spmd

    nc = _get_program()
    in_maps = _host_prep(inp, quant_weight, scales, zeros)
    res = run_bass_kernel_spmd(nc, in_maps, core_ids=list(range(NCORES)))
    sc = np.asarray(scales, dtype=np.float64).reshape(-1)
    parts = []
    for c in range(NCORES):
        r = res.results[c]["out"].astype(np.float64)  # [32, OS] hi/lo planes
        rows = slice(c * OS, (c + 1) * OS)
        parts.append((r[0:16] + r[16:32]) * (sc[rows] * 512.0)[None, :])
    out = np.concatenate(parts, axis=1)
    return np.ascontiguousarray(out.astype(np.float32))


# revision 3
# speedup vs baseline: 1.2966x; 1.2966x over previous
"""4-bit column-block-quantized linear on 8 TRN2 cores — fp8 DoubleRow, v2.

Math:  out[b,o] = scales[o] * (sum_i inp[b,i]*wq[o,i] - zeros[o]*rowsum[b])
where wq nibbles come from packed bytes q[o,j] (j = i//2): even i -> low
nibble, odd i -> high nibble.

Device scheme (all O(O*I) work on-device):
  * Packed bytes stream through the PE as float8e4: nibble bit patterns
    0x0..0xF ARE e4m3 values nibble*2^-9, so unpacking is 2 DVE
    tensor_scalar ops per chunk (uint32 views, 2x_2p mode):
        l = q & 0x0F0F0F0F ; h = (q >> 4) & 0x0F0F0F0F
    The 2^9 folds into the final host-side scales multiply.
  * fp8 DoubleRow matmuls: stationary = activations split hi/lo fp8
    (psum rows 0:16 hi, 16:32 lo), moving = the nibble streams.
  * -zeros*rowsum lands via a K=4 bf16 rank-1 matmul issued first, plus
    warm-up matmuls into a scratch psum bank so the PE p-state ramps
    while the weight DMAs stream in.
  * Drain: DVE copies psum[0:32] to SBUF, one DMA out; host adds the
    hi/lo planes and applies 512*scales.

Layout/overlap:
  * q repacked host-side to partition-contiguous [128, 22016B] so the
    weight stream needs only 5 big DMAs (1+1+2+2+2 dkt chunks), issued
    back-to-back on Sync while Scalar issues the const DMAs in parallel.
  * No buffer reuse anywhere (single-assignment tiles) to minimize
    semaphores and anti-dependency stalls.

Sharding: column-parallel over out_features (1376 rows/core), inputs
replicated; per-core output [2*16,1376] gathered+reduced on host.
"""

import numpy as np
import ml_dtypes

B = 16
I = 4096
O = 11008
NCORES = 8
OS = O // NCORES          # 1376 out-features per core
HALF = I // 2             # 2048 packed columns (j)
NDKT = 8                  # double-k-tiles of 256 j-rows each
BLKS = [(0, 512), (512, 512), (1024, 352)]  # psum-bank o-blocks
CHUNKS = [(0, 1), (1, 1), (2, 2), (4, 2), (6, 2)]  # q DMA chunks (d0, ndkt)
NWARM = 2                 # PE warm-up matmuls into scratch psum

BF16 = ml_dtypes.bfloat16
FP8 = ml_dtypes.float8_e4m3fn

_CACHE = {}


def _split_bf16(x64):
    hi = x64.astype(BF16)
    lo = (x64 - hi.astype(np.float64)).astype(BF16)
    return hi, lo


def _split_fp8(x64):
    hi = x64.astype(FP8)
    lo = (x64 - hi.astype(np.float64)).astype(FP8)
    return hi, lo


NWARM_PRE = 5             # zero-dependency warm-ups before the corr matmuls
NWARM_POST = 3            # warm-ups between corr and dkt0


def _build_program():
    import contextlib

    import concourse.bacc as bacc
    import concourse.mybir as mybir

    dt = mybir.dt
    op = mybir.AluOpType
    pm = mybir.MatmulPerfMode
    nc = bacc.Bacc("TRN2", target_bir_lowering=False)

    qa = nc.dram_tensor("qa", [128, NDKT * 688], dt.uint32, kind="ExternalInput")
    stat = nc.dram_tensor(
        "stat", [128, NDKT * 2 * 64], dt.float8e4, kind="ExternalInput"
    )
    corr = nc.dram_tensor("corr", [4, 32 + OS], dt.bfloat16, kind="ExternalInput")
    out_d = nc.dram_tensor("out", [32, OS], dt.float32, kind="ExternalOutput")

    ctx = contextlib.ExitStack()
    with ctx:
        sp_dma = ctx.enter_context(nc.semaphore("sp_dma"))
        sc_dma = ctx.enter_context(nc.semaphore("sc_dma"))
        dve_sem = ctx.enter_context(nc.semaphore("dve_sem"))
        pe_sem = ctx.enter_context(nc.semaphore("pe_sem"))

        stat_sb = ctx.enter_context(
            nc.sbuf_tensor("stat_sb", [128, NDKT * 128], dt.float8e4)
        )
        corr_sb = ctx.enter_context(
            nc.sbuf_tensor("corr_sb", [4, 32 + OS], dt.bfloat16)
        )
        # warm-up scratch: read uninitialized, result discarded in psum scratch
        scr = ctx.enter_context(nc.sbuf_tensor("scr", [4, 544], dt.bfloat16))
        qts, lbs, hbs = [], [], []
        for c, (d0, n) in enumerate(CHUNKS):
            qts.append(
                ctx.enter_context(nc.sbuf_tensor(f"qt{c}", [128, 688 * n], dt.uint32))
            )
            lbs.append(
                ctx.enter_context(nc.sbuf_tensor(f"lb{c}", [128, 688 * n], dt.uint32))
            )
            hbs.append(
                ctx.enter_context(nc.sbuf_tensor(f"hb{c}", [128, 688 * n], dt.uint32))
            )
        out_sb = ctx.enter_context(nc.sbuf_tensor("out_sb", [32, OS], dt.float32))

        psums = [
            ctx.enter_context(nc.psum_tensor(f"ps{i}", [32, n], dt.float32))
            for i, (s, n) in enumerate(BLKS)
        ]
        ps_w = ctx.enter_context(nc.psum_tensor("psw", [32, 512], dt.float32))

        corrL = corr_sb[:, 0:32]
        corrR = corr_sb[:, 32 : 32 + OS]

        def stat_ap(d, s):
            a = stat_sb[:, d * 128 + s * 64 : d * 128 + (s + 1) * 64]
            return a.rearrange("p (g m) -> p g m", g=2)

        with nc.Block() as block:

            @block.sync
            def _(sync):
                # tiny corr first (gates the first real matmuls), then weights
                sync.dma_start(corr_sb[:, :], corr[:, :]).then_inc(sp_dma, 16)
                for c, (d0, n) in enumerate(CHUNKS):
                    sync.dma_start(
                        qts[c][:, :], qa[:, d0 * 688 : (d0 + n) * 688]
                    ).then_inc(sp_dma, 16)
                # out: wait for the three DVE drains, then ship
                sync.wait_ge(dve_sem, 2 * len(CHUNKS) + len(BLKS))
                sync.dma_start(out_d[:, :], out_sb[:, :]).then_inc(sp_dma, 16)
                sync.wait_ge(sp_dma, 16 * (len(CHUNKS) + 2))

            @block.scalar
            def _(scalar):
                scalar.dma_start(stat_sb[:, :], stat[:, :]).then_inc(sc_dma, 16)

            @block.vector
            def _(vector):
                for c, (d0, n) in enumerate(CHUNKS):
                    vector.wait_ge(sp_dma, 16 * (c + 2))
                    vector.tensor_scalar(
                        lbs[c][:, :], qts[c][:, :], 0x0F0F0F0F, None, op.bitwise_and
                    ).then_inc(dve_sem)
                    vector.tensor_scalar(
                        hbs[c][:, :], qts[c][:, :], 4, 0x0F0F0F0F,
                        op.logical_shift_right, op.bitwise_and,
                    ).then_inc(dve_sem)
                for i, (s0, n) in enumerate(BLKS):
                    vector.wait_ge(pe_sem, i + 1)
                    # psum hi+lo rows -> SBUF; host adds the planes
                    vector.tensor_scalar(
                        out_sb[:, s0 : s0 + n], psums[i][:, :], 0.0, None, op.add
                    ).then_inc(dve_sem)

            @block.tensor
            def _(tensor):
                # zero-dep warm-ups: start the p-state ramp immediately
                for _ in range(NWARM_PRE):
                    tensor.matmul(
                        ps_w[:, :], scr[:, 0:32], scr[:, 32:544],
                        start=True, stop=True,
                    )
                tensor.wait_ge(sp_dma, 16)  # corr landed
                for i, (s0, n) in enumerate(BLKS):
                    tensor.matmul(
                        psums[i][:, :], corrL, corrR[:, s0 : s0 + n],
                        start=True, stop=False,
                    )
                for _ in range(NWARM_POST):
                    tensor.matmul(
                        ps_w[:, :], scr[:, 0:32], scr[:, 32:544],
                        start=True, stop=True,
                    )
                tensor.wait_ge(sc_dma, 16)  # stat landed
                for c, (d0, ndk) in enumerate(CHUNKS):
                    for t in range(ndk):
                        d = d0 + t
                        last = d == NDKT - 1
                        for s, buf in ((0, lbs[c]), (1, hbs[c])):
                            if t == 0:
                                tensor.wait_ge(dve_sem, 2 * c + 1 + s)
                            mv = (
                                buf[:, :]
                                .bitcast(dt.float8e4)[:, t * 2752 : (t + 1) * 2752]
                                .rearrange("p (o g) -> p g o", g=2)
                            )
                            sa = stat_ap(d, s)
                            for i, (s0, n) in enumerate(BLKS):
                                stop = last and s == 1
                                mm = tensor.matmul(
                                    psums[i][:, :], sa, mv[:, :, s0 : s0 + n],
                                    start=False, stop=stop, perf_mode=pm.DoubleRow,
                                )
                                if stop:
                                    mm.then_inc(pe_sem)

    nc.finalize()
    return nc


def _get_program():
    if "nc" not in _CACHE:
        _CACHE["nc"] = _build_program()
    return _CACHE["nc"]


def _host_prep(inp, quant_weight, scales, zeros):
    """Build per-core input maps (layout/precision prep, no dequant math)."""
    inp64 = np.asarray(inp, dtype=np.float64)
    a = np.ascontiguousarray(inp64[:, 0::2].T)  # [HALF, B] even-i (pairs l)
    b = np.ascontiguousarray(inp64[:, 1::2].T)  # [HALF, B] odd-i  (pairs h)
    a_hi, a_lo = _split_fp8(a)
    b_hi, b_lo = _split_fp8(b)

    def stream_stat(hi, lo):
        # [HALF,B] -> [NDKT,2,128,2B]: per dkt d, group g, j=d*256+g*128+p,
        # cols [hi(16) lo(16)]
        h = hi.reshape(NDKT, 2, 128, B)
        l = lo.reshape(NDKT, 2, 128, B)
        return np.concatenate([h, l], axis=-1)  # [d, g, p, 32]

    sa = stream_stat(a_hi, a_lo)  # stream 0: even i
    sb = stream_stat(b_hi, b_lo)  # stream 1: odd i
    st = np.stack([sa, sb], axis=1)  # [d, s, g, p, 32]
    stat_m = np.ascontiguousarray(
        st.transpose(3, 0, 1, 2, 4).reshape(128, NDKT * 2 * 2 * 32)
    )

    rowsum = inp64.sum(axis=1)  # [B]
    rs_hi, rs_lo = _split_bf16(rowsum)
    s9 = np.float64(2.0**-9)
    corrL = np.zeros((4, 32), dtype=BF16)
    corrL[0, :B] = (rs_hi.astype(np.float64) * s9).astype(BF16)
    corrL[1, :B] = corrL[0, :B]
    corrL[2, :B] = (rs_lo.astype(np.float64) * s9).astype(BF16)
    corrL[3, :B] = corrL[2, :B]

    qw = np.asarray(quant_weight)
    zeros = np.asarray(zeros, dtype=np.float64).reshape(-1)

    in_maps = []
    for cidx in range(NCORES):
        rows = slice(cidx * OS, (cidx + 1) * OS)
        qc = np.ascontiguousarray(qw[rows].astype(np.uint8).T)  # [HALF, OS]
        # byte (2o+g) of (dkt d, partition p) = qc[d*256+g*128+p, o];
        # partition-contiguous: partition p holds dkt d at byte offset d*2752
        q_arr = np.ascontiguousarray(
            qc.reshape(NDKT, 2, 128, OS)
            .transpose(2, 0, 3, 1)  # [p, d, o, g]
            .reshape(128, NDKT * 2 * OS)
        ).view(np.uint32)
        z_hi, z_lo = _split_bf16(zeros[rows])
        corr_m = np.zeros((4, 32 + OS), dtype=BF16)
        corr_m[:, :32] = corrL
        corr_m[0, 32:] = -z_hi
        corr_m[1, 32:] = -z_lo
        corr_m[2, 32:] = -z_hi
        corr_m[3, 32:] = -z_lo
        in_maps.append({"qa": q_arr, "stat": stat_m, "corr": corr_m})
    return in_maps


def kernel(inp, quant_weight, scales, zeros):
    from concourse.bass_utils import run_bass_kernel_spmd

    nc = _get_program()
    in_maps = _host_prep(inp, quant_weight, scales, zeros)
    res = run_bass_kernel_spmd(nc, in_maps, core_ids=list(range(NCORES)))
    sc = np.asarray(scales, dtype=np.float64).reshape(-1)
    parts = []
    for c in range(NCORES):
        r = res.results[c]["out"].astype(np.float64)  # [32, OS] hi/lo planes
        rows = slice(c * OS, (c + 1) * OS)
        parts.append((r[0:16] + r[16:32]) * (sc[rows] * 512.0)[None, :])
    out = np.concatenate(parts, axis=1)
    return np.ascontiguousarray(out.astype(np.float32))


# revision 5
# speedup vs baseline: 1.3402x; 1.0336x over previous
"""4-bit column-block-quantized linear on 8 TRN2 cores — fp8 DoubleRow, v2.

Math:  out[b,o] = scales[o] * (sum_i inp[b,i]*wq[o,i] - zeros[o]*rowsum[b])
where wq nibbles come from packed bytes q[o,j] (j = i//2): even i -> low
nibble, odd i -> high nibble.

Device scheme (all O(O*I) work on-device):
  * Packed bytes stream through the PE as float8e4: nibble bit patterns
    0x0..0xF ARE e4m3 values nibble*2^-9, so unpacking is 2 DVE
    tensor_scalar ops per chunk (uint32 views, 2x_2p mode):
        l = q & 0x0F0F0F0F ; h = (q >> 4) & 0x0F0F0F0F
    The 2^9 folds into the final host-side scales multiply.
  * fp8 DoubleRow matmuls: stationary = activations split hi/lo fp8
    (psum rows 0:16 hi, 16:32 lo), moving = the nibble streams.
  * -zeros*rowsum lands via a K=4 bf16 rank-1 matmul issued first, plus
    warm-up matmuls into a scratch psum bank so the PE p-state ramps
    while the weight DMAs stream in.
  * Drain: DVE copies psum[0:32] to SBUF, one DMA out; host adds the
    hi/lo planes and applies 512*scales.

Layout/overlap:
  * q repacked host-side to partition-contiguous [128, 22016B] so the
    weight stream needs only 5 big DMAs (1+1+2+2+2 dkt chunks), issued
    back-to-back on Sync while Scalar issues the const DMAs in parallel.
  * No buffer reuse anywhere (single-assignment tiles) to minimize
    semaphores and anti-dependency stalls.

Sharding: column-parallel over out_features (1376 rows/core), inputs
replicated; per-core output [2*16,1376] gathered+reduced on host.
"""

import numpy as np
import ml_dtypes

B = 16
I = 4096
O = 11008
NCORES = 8
OS = O // NCORES          # 1376 out-features per core
HALF = I // 2             # 2048 packed columns (j)
NDKT = 8                  # double-k-tiles of 256 j-rows each
BLKS = [(0, 512), (512, 512), (1024, 352)]  # psum-bank o-blocks
CHUNKS = [(0, 1), (1, 1), (2, 2), (4, 2), (6, 2)]  # q DMA chunks (d0, ndkt)
NWARM = 2                 # PE warm-up matmuls into scratch psum

BF16 = ml_dtypes.bfloat16
FP8 = ml_dtypes.float8_e4m3fn

_CACHE = {}


def _split_bf16(x64):
    hi = x64.astype(BF16)
    lo = (x64 - hi.astype(np.float64)).astype(BF16)
    return hi, lo


def _split_fp8(x64):
    hi = x64.astype(FP8)
    lo = (x64 - hi.astype(np.float64)).astype(FP8)
    return hi, lo


NWARM_PRE = 5             # zero-dependency warm-ups before the corr matmuls
NWARM_POST = 3            # warm-ups between corr and dkt0


def _build_program():
    import contextlib

    import concourse.bacc as bacc
    import concourse.mybir as mybir

    dt = mybir.dt
    op = mybir.AluOpType
    pm = mybir.MatmulPerfMode
    nc = bacc.Bacc("TRN2", target_bir_lowering=False)

    qa = nc.dram_tensor("qa", [128, NDKT * 688], dt.uint32, kind="ExternalInput")
    stat = nc.dram_tensor(
        "stat", [128, NDKT * 2 * 64], dt.float8e4, kind="ExternalInput"
    )
    corr = nc.dram_tensor("corr", [4, 32 + OS], dt.bfloat16, kind="ExternalInput")
    out_d = nc.dram_tensor("out", [32, OS], dt.float32, kind="ExternalOutput")

    ctx = contextlib.ExitStack()
    with ctx:
        sp_dma = ctx.enter_context(nc.semaphore("sp_dma"))
        sc_dma = ctx.enter_context(nc.semaphore("sc_dma"))
        dve_sem = ctx.enter_context(nc.semaphore("dve_sem"))
        pe_sem = ctx.enter_context(nc.semaphore("pe_sem"))

        stat_sb = ctx.enter_context(
            nc.sbuf_tensor("stat_sb", [128, NDKT * 128], dt.float8e4)
        )
        corr_sb = ctx.enter_context(
            nc.sbuf_tensor("corr_sb", [4, 32 + OS], dt.bfloat16)
        )
        # warm-up scratch: read uninitialized, result discarded in psum scratch
        scr = ctx.enter_context(nc.sbuf_tensor("scr", [4, 544], dt.bfloat16))
        qts, lbs, hbs = [], [], []
        for c, (d0, n) in enumerate(CHUNKS):
            qts.append(
                ctx.enter_context(nc.sbuf_tensor(f"qt{c}", [128, 688 * n], dt.uint32))
            )
            lbs.append(
                ctx.enter_context(nc.sbuf_tensor(f"lb{c}", [128, 688 * n], dt.uint32))
            )
            hbs.append(
                ctx.enter_context(nc.sbuf_tensor(f"hb{c}", [128, 688 * n], dt.uint32))
            )
        out_sb = ctx.enter_context(nc.sbuf_tensor("out_sb", [32, OS], dt.float32))

        psums = [
            ctx.enter_context(nc.psum_tensor(f"ps{i}", [32, n], dt.float32))
            for i, (s, n) in enumerate(BLKS)
        ]
        ps_w = ctx.enter_context(nc.psum_tensor("psw", [32, 512], dt.float32))

        corrL = corr_sb[:, 0:32]
        corrR = corr_sb[:, 32 : 32 + OS]

        def stat_ap(d, s):
            a = stat_sb[:, d * 128 + s * 64 : d * 128 + (s + 1) * 64]
            return a.rearrange("p (g m) -> p g m", g=2)

        with nc.Block() as block:

            @block.sync
            def _(sync):
                for c, (d0, n) in enumerate(CHUNKS):
                    sync.dma_start(
                        qts[c][:, :], qa[:, d0 * 688 : (d0 + n) * 688]
                    ).then_inc(sp_dma, 16)
                # wait for the three drains (DVE x2 + ACT, all tick dve_sem)
                sync.wait_ge(dve_sem, 2 * len(CHUNKS) + len(BLKS))
                # end-of-block Drain flushes in-flight DMAs; no completion wait
                # (walrus requires a sem update on every DMA)
                sync.dma_start(out_d[:, :], out_sb[:, :]).then_inc(sp_dma, 16)

            @block.scalar
            def _(scalar):
                scalar.dma_start(stat_sb[:, :], stat[:, :]).then_inc(sc_dma, 16)
                scalar.dma_start(corr_sb[:, :], corr[:, :]).then_inc(sc_dma, 32)
                # drain blk1 in parallel with DVE's blk0 drain
                scalar.wait_ge(pe_sem, 3)
                scalar.activation(
                    out_sb[:, BLKS[1][0] : BLKS[1][0] + BLKS[1][1]],
                    psums[1][:, :],
                    mybir.ActivationFunctionType.Copy,
                ).then_inc(dve_sem)

            @block.vector
            def _(vector):
                for c, (d0, n) in enumerate(CHUNKS):
                    vector.wait_ge(sp_dma, 16 * (c + 1))
                    vector.tensor_scalar(
                        lbs[c][:, :], qts[c][:, :], 0x0F0F0F0F, None, op.bitwise_and
                    ).then_inc(dve_sem)
                    vector.tensor_scalar(
                        hbs[c][:, :], qts[c][:, :], 4, 0x0F0F0F0F,
                        op.logical_shift_right, op.bitwise_and,
                    ).then_inc(dve_sem)
                # psum hi+lo rows -> SBUF; host adds the planes.
                # waits give one extra PE tick of margin past each block's stop
                for i, pe_tick in ((0, 2), (2, 5)):
                    s0, n = BLKS[i]
                    vector.wait_ge(pe_sem, pe_tick)
                    vector.tensor_scalar(
                        out_sb[:, s0 : s0 + n], psums[i][:, :], 0.0, None, op.add
                    ).then_inc(dve_sem)

            @block.tensor
            def _(tensor):
                # zero-dep warm-ups: keep the PE busy from t=0
                for _ in range(NWARM_PRE):
                    tensor.matmul(
                        ps_w[:, :], scr[:, 0:32], scr[:, 32:544],
                        start=True, stop=True,
                    )

                def dkt_matmuls(c, t, first=False):
                    d = CHUNKS[c][0] + t
                    last = d == NDKT - 1
                    for s, buf in ((0, lbs[c]), (1, hbs[c])):
                        if t == 0:
                            tensor.wait_ge(dve_sem, 2 * c + 1 + s)
                        mv = (
                            buf[:, :]
                            .bitcast(dt.float8e4)[:, t * 2752 : (t + 1) * 2752]
                            .rearrange("p (o g) -> p g o", g=2)
                        )
                        sa = stat_ap(d, s)
                        for i, (s0, n) in enumerate(BLKS):
                            stop = last and s == 1
                            mm = tensor.matmul(
                                psums[i][:, :], sa, mv[:, :, s0 : s0 + n],
                                start=first and s == 0, stop=stop,
                                perf_mode=pm.DoubleRow,
                            )
                            if stop:
                                mm.then_inc(pe_sem)

                # dkt0 opens the accumulation (start=True); the rank-1
                # correction joins after dkt0 — off the critical path
                tensor.wait_ge(sc_dma, 16)  # stat landed
                dkt_matmuls(0, 0, first=True)
                tensor.wait_ge(sc_dma, 32)  # corr landed
                for i, (s0, n) in enumerate(BLKS):
                    tensor.matmul(
                        psums[i][:, :], corrL, corrR[:, s0 : s0 + n],
                        start=False, stop=False,
                    )
                for c, (d0, ndk) in enumerate(CHUNKS):
                    for t in range(ndk):
                        if c == 0 and t == 0:
                            continue
                        dkt_matmuls(c, t)
                # trailing warm-ups: psum writeback margin for the drains
                for _ in range(2):
                    tensor.matmul(
                        ps_w[:, :], scr[:, 0:32], scr[:, 32:544],
                        start=True, stop=True,
                    ).then_inc(pe_sem)

    nc.finalize()
    return nc


def _get_program():
    if "nc" not in _CACHE:
        _CACHE["nc"] = _build_program()
    return _CACHE["nc"]


def _host_prep(inp, quant_weight, scales, zeros):
    """Build per-core input maps (layout/precision prep, no dequant math)."""
    inp64 = np.asarray(inp, dtype=np.float64)
    a = np.ascontiguousarray(inp64[:, 0::2].T)  # [HALF, B] even-i (pairs l)
    b = np.ascontiguousarray(inp64[:, 1::2].T)  # [HALF, B] odd-i  (pairs h)
    a_hi, a_lo = _split_fp8(a)
    b_hi, b_lo = _split_fp8(b)

    def stream_stat(hi, lo):
        # [HALF,B] -> [NDKT,2,128,2B]: per dkt d, group g, j=d*256+g*128+p,
        # cols [hi(16) lo(16)]
        h = hi.reshape(NDKT, 2, 128, B)
        l = lo.reshape(NDKT, 2, 128, B)
        return np.concatenate([h, l], axis=-1)  # [d, g, p, 32]

    sa = stream_stat(a_hi, a_lo)  # stream 0: even i
    sb = stream_stat(b_hi, b_lo)  # stream 1: odd i
    st = np.stack([sa, sb], axis=1)  # [d, s, g, p, 32]
    stat_m = np.ascontiguousarray(
        st.transpose(3, 0, 1, 2, 4).reshape(128, NDKT * 2 * 2 * 32)
    )

    rowsum = inp64.sum(axis=1)  # [B]
    rs_hi, rs_lo = _split_bf16(rowsum)
    s9 = np.float64(2.0**-9)
    corrL = np.zeros((4, 32), dtype=BF16)
    corrL[0, :B] = (rs_hi.astype(np.float64) * s9).astype(BF16)
    corrL[1, :B] = corrL[0, :B]
    corrL[2, :B] = (rs_lo.astype(np.float64) * s9).astype(BF16)
    corrL[3, :B] = corrL[2, :B]

    qw = np.asarray(quant_weight)
    zeros = np.asarray(zeros, dtype=np.float64).reshape(-1)

    in_maps = []
    for cidx in range(NCORES):
        rows = slice(cidx * OS, (cidx + 1) * OS)
        qc = np.ascontiguousarray(qw[rows].astype(np.uint8).T)  # [HALF, OS]
        # byte (2o+g) of (dkt d, partition p) = qc[d*256+g*128+p, o];
        # partition-contiguous: partition p holds dkt d at byte offset d*2752
        q_arr = np.ascontiguousarray(
            qc.reshape(NDKT, 2, 128, OS)
            .transpose(2, 0, 3, 1)  # [p, d, o, g]
            .reshape(128, NDKT * 2 * OS)
        ).view(np.uint32)
        z_hi, z_lo = _split_bf16(zeros[rows])
        corr_m = np.zeros((4, 32 + OS), dtype=BF16)
        corr_m[:, :32] = corrL
        corr_m[0, 32:] = -z_hi
        corr_m[1, 32:] = -z_lo
        corr_m[2, 32:] = -z_hi
        corr_m[3, 32:] = -z_lo
        in_maps.append({"qa": q_arr, "stat": stat_m, "corr": corr_m})
    return in_maps


def kernel(inp, quant_weight, scales, zeros):
    from concourse.bass_utils import run_bass_kernel_spmd

    nc = _get_program()
    in_maps = _host_prep(inp, quant_weight, scales, zeros)
    res = run_bass_kernel_spmd(nc, in_maps, core_ids=list(range(NCORES)))
    sc = np.asarray(scales, dtype=np.float64).reshape(-1)
    parts = []
    for c in range(NCORES):
        r = res.results[c]["out"].astype(np.float64)  # [32, OS] hi/lo planes
        rows = slice(c * OS, (c + 1) * OS)
        parts.append((r[0:16] + r[16:32]) * (sc[rows] * 512.0)[None, :])
    out = np.concatenate(parts, axis=1)
    return np.ascontiguousarray(out.astype(np.float32))
